# revision 2
# baseline (speedup 1.0000x reference)
"""Causal self-attention on 8 Trainium2 NeuronCores.

Sharding: core c = (batch b = c//2) x (head-half h2 = c%2). Each core
computes, for its batch and its 8 heads (of 16): the QKV projection
(only its W_qkv columns), causal flash attention, and a *partial*
output projection against its 512 rows of W_out. The host sums the
two half partials per batch and adds b_out. No device collectives.

On-device layout (per core):
  xT   (1024, 2048)  x[b] transposed (host-side, free)
  Q^T/K^T (64, T) per head   -- from W-stationary matmuls (qkv^T comes
                                out with channel on partitions)
  V    (T, 64) per head, with a fused ones-column (65 cols) so the
       P@V matmul also emits the softmax denominator row.
  S^T  (k-part, q-free) blocks -> exp on ScalarE (scale=1/8 fused,
       causal mask added only on diagonal 128x128 blocks)
  O'^T (65, q) accumulated in PSUM over k-blocks; row 64 = denom.
  Normalize on VectorE with a GpSimd partition-broadcast reciprocal.
  Out-proj: lhsT = paired-head O^T chunks, rhs = W_out rows.

All matmuls run in float32r (TF32-like, 1 cyc/row at N>=256).
"""
import os
import sys

sys.path.insert(0, "/opt/trn_rl_repo")

import numpy as np

import concourse.bacc as bacc
import concourse.mybir as mybir
import concourse.tile as tile
from concourse.bass_utils import run_bass_kernel_spmd

B, T, C = 4, 2048, 1024
H = 16
HD = C // H              # 64
N_CORES = 8
HL = H // 2              # 8 local heads per core
CL = HL * HD             # 512 local channels
F32 = mybir.dt.float32
F32R = mybir.dt.float32r

QG = 1024                # q-group width in phase 2
NQG = T // QG            # 2
KB = 128                 # k-block
NKB = T // KB            # 16
TCH = 128                # t-chunk (tokens per matmul M)
NTCH = T // TCH          # 16
CCH = 128                # channel chunk (contraction tile)
NCCH = C // CCH          # 8

_cache = {}


def _build(dbg=False, reps=1):
    nc = bacc.Bacc("TRN2", target_bir_lowering=False, debug=False,
                   num_devices=N_CORES)

    xT = nc.dram_tensor("xT", [C, T], F32R, kind="ExternalInput")
    wqk = nc.dram_tensor("wqk", [C, 2 * CL], F32R, kind="ExternalInput")
    wv = nc.dram_tensor("wv", [C, CL], F32R, kind="ExternalInput")
    wout = nc.dram_tensor("wout", [CL, C], F32R, kind="ExternalInput")
    BF16 = mybir.dt.bfloat16
    mask = nc.dram_tensor("mask", [KB, KB], BF16, kind="ExternalInput")
    ident = nc.dram_tensor("ident", [KB, KB], BF16, kind="ExternalInput")
    y = nc.dram_tensor("y", [T, C], F32, kind="ExternalOutput")
    if dbg:
        d_qk0 = nc.dram_tensor("d_qk0", [128, T], F32, kind="ExternalOutput")
        d_qk4 = nc.dram_tensor("d_qk4", [128, T], F32, kind="ExternalOutput")
        d_vw0 = nc.dram_tensor("d_vw0", [128, HL * (HD + 1)], F32,
                               kind="ExternalOutput")
        d_p = nc.dram_tensor("d_p", [128, QG], F32, kind="ExternalOutput")
        d_rr = nc.dram_tensor("d_rr", [1, QG], F32, kind="ExternalOutput")
        d_rb = nc.dram_tensor("d_rb", [64, QG], F32, kind="ExternalOutput")
        d_ot0 = nc.dram_tensor("d_ot0", [128, T], F32, kind="ExternalOutput")

    with tile.TileContext(nc) as tc:
      for _rep in range(reps):
        with tc.tile_pool(name="persist", bufs=1) as pp:
            # persistent SBUF tiles
            qk = [pp.tile([128, T], F32R, tag=f"qk{j}", name=f"qk{j}") for j in range(8)]
            #   qk[0..3] = Q^T pairs (head 2j at part 0-63, 2j+1 at 64-127)
            #   qk[4..7] = K^T pairs
            vws = [pp.tile([128, HL * (HD + 1)], F32R, tag=f"vw{m}", name=f"vw{m}")
                   for m in range(NTCH)]      # V' tiles: (128t, 8*(64+1))
            ot = [pp.tile([128, T], F32R, tag=f"ot{j}", name=f"ot{j}") for j in range(4)]
            BF16 = mybir.dt.bfloat16
            msk = pp.tile([KB, KB], BF16, tag="msk", name="msk")
            nc.sync.dma_start(msk[:], mask[:])
            idn = pp.tile([KB, KB], BF16, tag="idn", name="idn")
            nc.sync.dma_start(idn[:], ident[:])

            # ---------------- Phase 1: QKV projection ----------------
            with (
                tc.tile_pool(name="p1", bufs=1) as p1,
                tc.tile_pool(name="p1w", bufs=2) as p1w,
                tc.tile_pool(name="ps_mm", bufs=4, space="PSUM") as ps_mm,
            ):
                wv_t = []
                for i in range(NCCH):
                    wv_i = p1.tile([128, CL], F32R, tag=f"wv{i}", name=f"wv{i}")
                    nc.sync.dma_start(
                        wv_i[:], wv[i * CCH:(i + 1) * CCH, :])
                    wv_t.append(wv_i)
                for half in range(2):
                    t0 = half * (T // 2)
                    xt = []
                    for i in range(NCCH):
                        xti = p1.tile([128, T // 2], F32R, tag=f"xt{i}", name=f"xt{i}")
                        nc.sync.dma_start(
                            xti[:], xT[i * CCH:(i + 1) * CCH, t0:t0 + T // 2])
                        xt.append(xti)

                    # Q^T / K^T: lhsT = wqk chunk (stationary, reused
                    # across the 2 t-groups of this half), rhs = xT.
                    for j in range(8):          # c' 128-chunks of [Q|K]
                        wc = []
                        for i in range(NCCH):
                            wci = p1w.tile([128, 128], F32R,
                                           tag=f"wc{i}", name=f"wc{i}")
                            nc.sync.dma_start(
                                wci[:], wqk[i * CCH:(i + 1) * CCH,
                                            j * 128:(j + 1) * 128])
                            wc.append(wci)
                        for tg in range(2):     # 512-wide t-groups
                            ps = ps_mm.tile([128, 512], F32, tag="mm", name="mm")
                            for i in range(NCCH):
                                nc.tensor.matmul(
                                    ps[:],
                                    wc[i][:],
                                    xt[i][:, tg * 512:(tg + 1) * 512],
                                    start=(i == 0), stop=(i == NCCH - 1))
                            dst = qk[j][:, t0 + tg * 512: t0 + (tg + 1) * 512]
                            nc.vector.tensor_copy(dst, ps[:])

                    # V: lhsT = xT t-chunk, rhs = wv columns.
                    for m in range(NTCH // 2):  # t-chunks in this half
                        ps = ps_mm.tile([128, CL], F32, tag="mm", name="mmv")
                        for i in range(NCCH):
                            nc.tensor.matmul(
                                ps[:],
                                xt[i][:, m * TCH:(m + 1) * TCH],
                                wv_t[i][:],
                                start=(i == 0), stop=(i == NCCH - 1))
                        vt = vws[half * (NTCH // 2) + m]
                        # scatter (h,d) -> (h, d | ones) layout
                        dst = vt[:].rearrange("p (h x) -> p h x", x=HD + 1)
                        nc.vector.tensor_copy(
                            dst[:, :, 0:HD],
                            ps[:].rearrange("p (h d) -> p h d", d=HD))
                        nc.vector.memset(dst[:, :, HD:HD + 1].bitcast(F32), 1.0)

            if dbg:
                nc.sync.dma_start(d_qk0[:], qk[0][:].bitcast(F32))
                nc.sync.dma_start(d_qk4[:], qk[4][:].bitcast(F32))
                nc.sync.dma_start(d_vw0[:], vws[0][:].bitcast(F32))

            # ---------------- Phase 2: attention ----------------
            with (
                tc.tile_pool(name="ps_s", bufs=2, space="PSUM") as ps_s,
                tc.tile_pool(name="ps_o", bufs=2, space="PSUM") as ps_o,
                tc.tile_pool(name="p2", bufs=3) as p2,
                tc.tile_pool(name="p2n", bufs=2) as p2n,
            ):
                for h in range(HL):
                    jp = h // 2               # pair index
                    pb = (h % 2) * 64         # partition base within pair
                    for g in range(NQG):
                        qlo = g * QG
                        nkb = (qlo + QG) // KB
                        o_ps = ps_o.tile([128, QG], F32, tag="ops", name="ops")

                        def emit_s(kb):
                            # S^T block: lhsT = K^T slice, rhs = Q^T; the
                            # causal mask on the diagonal is accumulated by
                            # a second (identity-weighted) matmul so the
                            # PSUM->exp chain stays PE->ACT with no DVE hop.
                            r0 = max(0, kb * KB - qlo)
                            s_ps = ps_s.tile([128, QG], F32, tag="sps",
                                             name="sps")
                            diag = kb * KB >= qlo
                            lhs = qk[4 + jp][pb:pb + 64,
                                             kb * KB:(kb + 1) * KB]
                            c0 = r0
                            while c0 < QG:
                                c1 = min(QG, (c0 // 512 + 1) * 512)
                                last = (not diag) or (c0 > r0) or (KB > c1 - c0)
                                nc.tensor.matmul(
                                    s_ps[:, c0:c1],
                                    lhs,
                                    qk[jp][pb:pb + 64, qlo + c0:qlo + c1],
                                    start=True,
                                    stop=(not diag) or (c0 != r0))
                                c0 = c1
                            if diag:
                                nc.tensor.matmul(
                                    s_ps[:, r0:r0 + KB], idn[:], msk[:],
                                    start=False, stop=True)
                            p_sb = p2.tile([128, QG], F32R, tag="p", name="p")
                            nc.scalar.activation(
                                p_sb[:, r0:], s_ps[:, r0:],
                                mybir.ActivationFunctionType.Exp,
                                scale=0.125)
                            if dbg and h == 0 and g == 0 and kb == 0:
                                nc.sync.dma_start(d_p[:],
                                                  p_sb[:].bitcast(F32))
                            return p_sb

                        def emit_pv(kb, p_sb):
                            # P@V' accumulate: out rows 0..64 (row 64 =
                            # softmax denominator via the ones column)
                            r0 = max(0, kb * KB - qlo)
                            lhv = vws[kb][:, h * (HD + 1):
                                          (h + 1) * (HD + 1)]
                            c0 = (r0 // 512) * 512
                            while c0 < QG:
                                c1 = min(QG, c0 + 512)
                                rs = max(c0, r0)
                                last_kb = min(nkb, (qlo + c1) // KB) - 1
                                nc.tensor.matmul(
                                    o_ps[0:HD + 1, rs:c1],
                                    lhv,
                                    p_sb[:, rs:c1],
                                    start=(kb == 0), stop=(kb == last_kb))
                                c0 = c1

                        prev = None
                        for kb in range(nkb):
                            p_sb = emit_s(kb)
                            if prev is not None:
                                emit_pv(*prev)
                            prev = (kb, p_sb)
                        emit_pv(*prev)
                        # normalize: recip of denom row, broadcast, mul.
                        # reciprocal is lane-locked (DVE), so it lands on
                        # partition 64; partition_broadcast reads physical
                        # partition 0, so DMA-hop the row down first.
                        rr = p2n.tile([65, QG], F32, tag="rr", name="rr")
                        nc.vector.reciprocal(rr[64:65, :], o_ps[HD:HD + 1, :])
                        rr0 = p2n.tile([1, QG], F32, tag="rr0", name="rr0")
                        nc.sync.dma_start(rr0[:], rr[64:65, :])
                        rb = p2n.tile([64, QG], F32, tag="rb", name="rb")
                        nc.gpsimd.partition_broadcast(rb[:], rr0[:])
                        if dbg and h == 0 and g == 0:
                            nc.sync.dma_start(d_rr[:], rr[64:65, :])
                            nc.sync.dma_start(d_rb[:], rb[:])
                        if pb == 0:
                            nc.vector.tensor_mul(
                                ot[jp][0:64, qlo:qlo + QG],
                                o_ps[0:HD, :], rb[:])
                        else:
                            os_ = p2n.tile([64, QG], F32R, tag="os", name="os")
                            nc.vector.tensor_mul(os_[:], o_ps[0:HD, :], rb[:])
                            nc.sync.dma_start(
                                ot[jp][64:128, qlo:qlo + QG], os_[:])

            if dbg:
                nc.sync.dma_start(d_ot0[:], ot[0][:].bitcast(F32))

            # ---------------- Phase 3: output projection ----------------
            with (
                tc.tile_pool(name="p3", bufs=2) as p3,
                tc.tile_pool(name="p3w", bufs=1) as p3w,
                tc.tile_pool(name="ps_mm", bufs=4, space="PSUM") as ps_mm,
            ):
                wo_t = []
                for j in range(4):
                    wj = p3w.tile([128, C], F32R, tag=f"wo{j}", name=f"wo{j}")
                    nc.sync.dma_start(wj[:], wout[j * 128:(j + 1) * 128, :])
                    wo_t.append(wj)
                for m in range(NTCH):
                    for n in range(2):
                        ps = ps_mm.tile([128, 512], F32, tag="mm", name="mm")
                        for j in range(4):
                            nc.tensor.matmul(
                                ps[:],
                                ot[j][:, m * TCH:(m + 1) * TCH],
                                wo_t[j][:, n * 512:(n + 1) * 512],
                                start=(j == 0), stop=(j == 3))
                        ysb = p3.tile([128, 512], F32, tag="y", name="y")
                        nc.scalar.copy(ysb[:], ps[:])
                        nc.sync.dma_start(
                            y[m * TCH:(m + 1) * TCH, n * 512:(n + 1) * 512],
                            ysb[:])

    nc.compile()
    return nc


def make_in_maps(x, W_qkv, W_out):
    x = np.asarray(x, dtype=np.float32)
    W_qkv = np.asarray(W_qkv, dtype=np.float32)
    W_out = np.asarray(W_out, dtype=np.float32)

    import ml_dtypes
    mask = np.where(
        np.arange(KB)[None, :] < np.arange(KB)[:, None], -1e30, 0.0
    ).astype(ml_dtypes.bfloat16)
    ident = np.eye(KB).astype(ml_dtypes.bfloat16)

    in_maps = []
    for c in range(N_CORES):
        b, h2 = c // 2, c % 2
        cols = slice(h2 * CL, (h2 + 1) * CL)
        in_maps.append({
            "xT": np.ascontiguousarray(x[b].T),
            "wqk": np.ascontiguousarray(
                np.concatenate([W_qkv[:, cols],
                                W_qkv[:, C:][:, cols]], axis=1)),
            "wv": np.ascontiguousarray(W_qkv[:, 2 * C:][:, cols]),
            "wout": np.ascontiguousarray(W_out[cols, :]),
            "mask": mask,
            "ident": ident,
        })
    return in_maps


def kernel(x, W_qkv, b_qkv, W_out, b_out, _trace=False):
    b_qkv = np.asarray(b_qkv, dtype=np.float32)
    b_out = np.asarray(b_out, dtype=np.float32)

    # q/k biases would need device-side adds; this problem pins them to 0.
    assert not b_qkv[:2 * C].any(), "nonzero q/k bias unsupported"

    if "nc" not in _cache:
        _cache["nc"] = _build()
    nc = _cache["nc"]

    in_maps = make_in_maps(x, W_qkv, W_out)

    kwargs = {}
    if _trace:
        kwargs = {"trace": True, "trace_cores": [0]}
    res = run_bass_kernel_spmd(nc, in_maps, core_ids=list(range(N_CORES)),
                               **kwargs)

    out = np.empty((B, T, C), dtype=np.float32)
    # v-bias passes through softmax as +b_v, so it folds into the output
    # projection; b_out likewise. Both are host-side adds on the partials.
    bias = b_qkv[2 * C:] @ W_out + b_out
    for b in range(B):
        out[b] = res.results[2 * b]["y"] + res.results[2 * b + 1]["y"] + bias
    if _trace:
        kernel.last_exec_ns = res.exec_time_ns
        kernel.last_trace = (res.instructions_and_trace or (None, None))[1]
    return out



# revision 7
# speedup vs baseline: 1.2979x; 1.2979x over previous
"""Causal self-attention on 8 Trainium2 NeuronCores.

Sharding: core c = (batch b = c//2) x (head-half h2 = c%2). Each core
computes, for its batch and its 8 heads (of 16): the QKV projection
(only its W_qkv columns), causal attention, and a *partial* output
projection against its 512 rows of W_out. The host sums the two half
partials per batch and adds b_out. No device collectives.

v2 design (all matmuls bf16):
  - Inputs land in SBUF via a handful of large DMAs (the HWDGE lock is
    ~625ns per dma_start, so descriptor count is minimized).
  - QKV projection runs in 512-token quarters; quarters 1-3 are
    interleaved into the attention loop to fill PE gaps (attention is
    ACT/exp-bound).
  - S^T blocks [k=128, q<=512] via K^T-stationary matmuls; the two heads
    of a pair occupy PE rows 0-63 / 64-127 and run CONCURRENTLY
    (row-tiling via automatic tile_position from partition offsets).
  - exp on ScalarE (scale=1/8 fused) -> bf16 P^T in SBUF. Diagonal
    blocks are masked AFTER exp by a lower-triangle 0/1 bf16 multiply
    on the DVE (keeps PE free, exact same zeros as a -inf mask).
  - PV uses P-stationary matmuls: lhsT = P^T block [k=128, q=128],
    rhs = V' [k=128, 65] (64 V cols + ones col). Output lands
    [q-partitions, d | denom] in PSUM, so the softmax denominator is a
    per-partition scalar: reciprocal + tensor_scalar_mul on DVE/Pool,
    no partition broadcasts or DMA hops.
  - O [q,d] bf16 is transposed back to O^T [d,q] pair-tiles with PE
    transposes; head A targets PSUM partitions 0-63, head B 64-127
    (col-tiling), so one DVE copy refills the pair tile.
  - Output projection per 512-token group is emitted right after that
    group's attention, overlapping the next group's work.
"""
import os
import sys

sys.path.insert(0, "/opt/trn_rl_repo")

import numpy as np

import concourse.bacc as bacc
import concourse.mybir as mybir
import concourse.tile as tile
from concourse.bass_utils import run_bass_kernel_spmd

B, T, C = 4, 2048, 1024
H = 16
HD = C // H              # 64
N_CORES = 8
HL = H // 2              # 8 local heads per core
CL = HL * HD             # 512 local channels
F32 = mybir.dt.float32
BF16 = mybir.dt.bfloat16

QG = 512                 # q-group width (4 groups)
NQG = T // QG            # 4
KB = 128                 # k-block
NTCH = T // 128          # 16 t-chunks (V' tiles)
NCCH = C // 128          # 8 contraction chunks

_cache = {}


def _build(dbg=False, reps=1):
    nc = bacc.Bacc("TRN2", target_bir_lowering=False, debug=False,
                   num_devices=N_CORES)

    xT = nc.dram_tensor("xT", [C, T], BF16, kind="ExternalInput")
    wqk = nc.dram_tensor("wqk", [C, 2 * CL], BF16, kind="ExternalInput")
    wv = nc.dram_tensor("wv", [C, CL], BF16, kind="ExternalInput")
    wout = nc.dram_tensor("wout", [CL, C], BF16, kind="ExternalInput")
    tmask = nc.dram_tensor("tmask", [KB, KB], BF16, kind="ExternalInput")
    ident = nc.dram_tensor("ident", [KB, KB], BF16, kind="ExternalInput")
    y = nc.dram_tensor("y", [T, C], F32, kind="ExternalOutput")

    with tile.TileContext(nc) as tc:
      for _rep in range(reps):
        with tc.tile_pool(name="persist", bufs=1) as pp:
            # ---- persistent SBUF tiles ----
            qk = [pp.tile([128, T], BF16, tag=f"qk{j}", name=f"qk{j}")
                  for j in range(8)]
            #   qk[0..3] = Q^T pairs (head 2j at parts 0-63, 2j+1 at 64-127)
            #   qk[4..7] = K^T pairs
            vws = [pp.tile([128, HL * (HD + 1)], BF16, tag=f"vw{m}",
                           name=f"vw{m}") for m in range(NTCH)]
            ot = [pp.tile([128, T], BF16, tag=f"ot{j}", name=f"ot{j}")
                  for j in range(4)]
            wo = [pp.tile([128, C], BF16, tag=f"wo{j}", name=f"wo{j}")
                  for j in range(4)]
            tm = pp.tile([KB, KB], BF16, tag="tm", name="tm")
            idn = pp.tile([KB, KB], BF16, tag="idn", name="idn")
            # phase-1 input tiles (single big DMAs, chunk-sliced in SBUF)
            xt = pp.tile([128, NCCH * T], BF16, tag="xt", name="xt")
            wq = pp.tile([128, NCCH * 2 * CL], BF16, tag="wq", name="wq")
            wvt = pp.tile([128, NCCH * CL], BF16, tag="wvt", name="wvt")

            nc.sync.dma_start(tm[:], tmask[:])
            nc.sync.dma_start(idn[:], ident[:])
            nc.sync.dma_start(
                xt[:].rearrange("p (i t) -> p i t", i=NCCH),
                xT[:].rearrange("(i p) t -> p i t", i=NCCH))
            nc.sync.dma_start(
                wq[:].rearrange("p (i n) -> p i n", i=NCCH),
                wqk[:].rearrange("(i p) n -> p i n", i=NCCH))
            nc.sync.dma_start(
                wvt[:].rearrange("p (i n) -> p i n", i=NCCH),
                wv[:].rearrange("(i p) n -> p i n", i=NCCH))
            nc.sync.dma_start(
                wo[0][:], wout[0 * 128:1 * 128, :])
            nc.sync.dma_start(
                wo[1][:], wout[1 * 128:2 * 128, :])
            nc.sync.dma_start(
                wo[2][:], wout[2 * 128:3 * 128, :])
            nc.sync.dma_start(
                wo[3][:], wout[3 * 128:4 * 128, :])

            def xt_s(i, t0, t1):
                return xt[:, i * T + t0: i * T + t1]

            def wq_s(i, c0, c1):
                return wq[:, i * 2 * CL + c0: i * 2 * CL + c1]

            def wv_s(i):
                return wvt[:, i * CL:(i + 1) * CL]

            with (
                tc.tile_pool(name="ps_mm", bufs=2, space="PSUM") as ps_mm,
                tc.tile_pool(name="ps_s", bufs=3, space="PSUM") as ps_s,
                tc.tile_pool(name="ps_o", bufs=2, space="PSUM") as ps_o,
                tc.tile_pool(name="ps_t", bufs=1, space="PSUM") as ps_t,
                tc.tile_pool(name="pb", bufs=34) as pb,
                tc.tile_pool(name="pn", bufs=4) as pn,
                tc.tile_pool(name="py", bufs=2) as py,
            ):
                cp_eng = [nc.vector, nc.gpsimd]

                def p1_qk(tq, j):
                    # Q^T/K^T columns j*128..+128 for tokens tq*512..+512
                    t0 = tq * QG
                    ps = ps_mm.tile([128, QG], F32, tag="mm", name="mm")
                    for i in range(NCCH):
                        nc.tensor.matmul(
                            ps[:], wq_s(i, j * 128, (j + 1) * 128),
                            xt_s(i, t0, t0 + QG),
                            start=(i == 0), stop=(i == NCCH - 1))
                    nc.vector.tensor_copy(qk[j][:, t0:t0 + QG], ps[:])

                def p1_v(tq, m):
                    # V' tile for t-chunk m (128 tokens) of quarter tq
                    mm = tq * 4 + m
                    t0 = mm * 128
                    ps = ps_mm.tile([128, CL], F32, tag="mm", name="mmv")
                    for i in range(NCCH):
                        nc.tensor.matmul(
                            ps[:], xt_s(i, t0, t0 + 128), wv_s(i),
                            start=(i == 0), stop=(i == NCCH - 1))
                    vt = vws[mm][:].rearrange("p (h x) -> p h x", x=HD + 1)
                    nc.vector.tensor_copy(
                        vt[:, :, 0:HD],
                        ps[:].rearrange("p (h d) -> p h d", d=HD))
                    nc.vector.memset(vt[:, :, HD:HD + 1], 1.0)

                def p1_quarter(tq):
                    for j in range(8):
                        p1_qk(tq, j)
                    for m in range(4):
                        p1_v(tq, m)

                def p1_slice(tq, pair):
                    p1_qk(tq, 2 * pair)
                    p1_qk(tq, 2 * pair + 1)
                    p1_v(tq, pair)

                def attn(g, pair):
                    qlo = g * QG
                    nkb = 4 * g + 4
                    # ---- S + exp for every k-block (P tiles kept live) ----
                    p_all = [[], []]
                    for kb in range(nkb):
                        r0 = max(0, kb * KB - qlo)
                        for x in range(2):   # head A (parts 0-63) / B
                            pb0 = x * 64
                            s_ps = ps_s.tile([128, QG], F32, tag="sps",
                                             name=f"sps{x}")
                            nc.tensor.matmul(
                                s_ps[:, r0:QG],
                                qk[4 + pair][pb0:pb0 + 64,
                                             kb * KB:(kb + 1) * KB],
                                qk[pair][pb0:pb0 + 64,
                                         qlo + r0:qlo + QG],
                                start=True, stop=True)
                            p_x = pb.tile([128, QG], BF16, tag="p",
                                          name=f"p{x}k{kb}")
                            nc.scalar.activation(
                                p_x[:, r0:QG], s_ps[:, r0:QG],
                                mybir.ActivationFunctionType.Exp,
                                scale=0.125)
                            if kb >= 4 * g:   # diagonal block: mask
                                cp_eng[(kb + x) % 2].tensor_mul(
                                    p_x[:, r0:r0 + KB],
                                    p_x[:, r0:r0 + KB], tm[:])
                            p_all[x].append(p_x)
                    # ---- PV' per q-block: one accumulation group at a
                    # time per PSUM bank (sequential starts), normalize +
                    # transpose between groups ----
                    o_ps = [ps_o.tile([128, 4 * (HD + 1)], F32, tag="ops",
                                      name=f"ops{x}") for x in range(2)]
                    o3 = [o[:].rearrange("p (q e) -> p q e", e=HD + 1)
                          for o in o_ps]
                    tp = ps_t.tile([128, QG], BF16, tag="tp", name="tp")
                    rcp = [pn.tile([128, 4], F32, tag="rcp", name=f"rcp{x}")
                           for x in range(2)]
                    ob = [pn.tile([128, 4 * HD], BF16, tag="ob",
                                  name=f"ob{x}") for x in range(2)]
                    for qb in range(4):
                        for x in range(2):
                            h = 2 * pair + x
                            for kb in range(4 * g + qb + 1):
                                vsl = vws[kb][:, h * (HD + 1):
                                              (h + 1) * (HD + 1)]
                                nc.tensor.matmul(
                                    o3[x][:, qb, :],
                                    p_all[x][kb][:, qb * KB:(qb + 1) * KB],
                                    vsl,
                                    start=(kb == 0),
                                    stop=(kb == 4 * g + qb))
                        for x in range(2):
                            nc.vector.reciprocal(
                                rcp[x][:, qb:qb + 1],
                                o3[x][:, qb, HD:HD + 1])
                            nc.vector.tensor_scalar_mul(
                                ob[x][:, qb * HD:(qb + 1) * HD],
                                o3[x][:, qb, 0:HD],
                                rcp[x][:, qb:qb + 1])
                            nc.tensor.transpose(
                                tp[x * 64:x * 64 + 64,
                                   qb * KB:(qb + 1) * KB],
                                ob[x][:, qb * HD:(qb + 1) * HD],
                                idn[:])
                    nc.vector.tensor_copy(ot[pair][:, qlo:qlo + QG], tp[:])

                def proj(g):
                    qlo = g * QG
                    ys = py.tile([128, 4 * C], F32, tag="ys", name="ys")
                    for tc_ in range(4):
                        t0 = qlo + tc_ * 128
                        for n in range(2):
                            ps = ps_mm.tile([128, 512], F32, tag="mm",
                                            name="mmp")
                            for j in range(4):
                                nc.tensor.matmul(
                                    ps[:],
                                    ot[j][:, t0:t0 + 128],
                                    wo[j][:, n * 512:(n + 1) * 512],
                                    start=(j == 0), stop=(j == 3))
                            nc.vector.tensor_copy(
                                ys[:, tc_ * C + n * 512:
                                   tc_ * C + (n + 1) * 512], ps[:])
                    nc.sync.dma_start(
                        y[qlo:qlo + QG, :].rearrange(
                            "(tc p) c -> p tc c", tc=4),
                        ys[:].rearrange("p (tc c) -> p tc c", tc=4))

                # ---------------- schedule ----------------
                p1_quarter(0)
                for g in range(NQG):
                    for pair in range(4):
                        attn(g, pair)
                        if g < NQG - 1:
                            p1_slice(g + 1, pair)
                    proj(g)

    nc.compile()
    return nc


def make_in_maps(x, W_qkv, W_out):
    import ml_dtypes
    bf16 = ml_dtypes.bfloat16
    x = np.asarray(x, dtype=np.float32)
    W_qkv = np.asarray(W_qkv, dtype=np.float32)
    W_out = np.asarray(W_out, dtype=np.float32)

    # lower-triangle keep-mask for the diagonal S^T blocks (rows=k, cols=q:
    # keep k<=q) and the identity for PE transposes
    tmask = (np.arange(KB)[:, None] <= np.arange(KB)[None, :]).astype(bf16)
    ident = np.eye(KB).astype(bf16)

    in_maps = []
    for c in range(N_CORES):
        b, h2 = c // 2, c % 2
        cols = slice(h2 * CL, (h2 + 1) * CL)
        in_maps.append({
            "xT": np.ascontiguousarray(x[b].T).astype(bf16),
            "wqk": np.ascontiguousarray(
                np.concatenate([W_qkv[:, cols],
                                W_qkv[:, C:][:, cols]], axis=1)).astype(bf16),
            "wv": np.ascontiguousarray(W_qkv[:, 2 * C:][:, cols]).astype(bf16),
            "wout": np.ascontiguousarray(W_out[cols, :]).astype(bf16),
            "tmask": tmask,
            "ident": ident,
        })
    return in_maps


def kernel(x, W_qkv, b_qkv, W_out, b_out, _trace=False):
    b_qkv = np.asarray(b_qkv, dtype=np.float32)
    b_out = np.asarray(b_out, dtype=np.float32)
    W_out_f = np.asarray(W_out, dtype=np.float32)

    # q/k biases would need device-side adds; this problem pins them to 0.
    assert not b_qkv[:2 * C].any(), "nonzero q/k bias unsupported"

    if "nc" not in _cache:
        _cache["nc"] = _build()
    nc = _cache["nc"]

    in_maps = make_in_maps(x, W_qkv, W_out)

    kwargs = {}
    if _trace:
        kwargs = {"trace": True, "trace_cores": [0]}
    res = run_bass_kernel_spmd(nc, in_maps, core_ids=list(range(N_CORES)),
                               **kwargs)

    out = np.empty((B, T, C), dtype=np.float32)
    # v-bias passes through softmax as +b_v, so it folds into the output
    # projection; b_out likewise. Both are host-side adds on the partials.
    bias = b_qkv[2 * C:] @ W_out_f + b_out
    for b in range(B):
        out[b] = res.results[2 * b]["y"] + res.results[2 * b + 1]["y"] + bias
    if _trace:
        kernel.last_exec_ns = res.exec_time_ns
        kernel.last_trace = (res.instructions_and_trace or (None, None))[1]
    return out


# revision 8
# speedup vs baseline: 4.4309x; 3.4140x over previous
"""Causal self-attention on 8 Trainium2 NeuronCores.

Sharding: core c = (batch b = c//2) x (head-half h2 = c%2). Each core
computes, for its batch and its 8 heads (of 16): the QKV projection
(only its W_qkv columns), causal attention, and a *partial* output
projection against its 512 rows of W_out. The host sums the two half
partials per batch and adds b_out. No device collectives.

v2 design (all matmuls bf16):
  - Inputs land in SBUF via a handful of large DMAs (the HWDGE lock is
    ~625ns per dma_start, so descriptor count is minimized).
  - QKV projection runs in 512-token quarters; quarters 1-3 are
    interleaved into the attention loop to fill PE gaps (attention is
    ACT/exp-bound).
  - S^T blocks [k=128, q<=512] via K^T-stationary matmuls; the two heads
    of a pair occupy PE rows 0-63 / 64-127 and run CONCURRENTLY
    (row-tiling via automatic tile_position from partition offsets).
  - exp on ScalarE (scale=1/8 fused) -> bf16 P^T in SBUF. Diagonal
    blocks are masked AFTER exp by a lower-triangle 0/1 bf16 multiply
    on the DVE (keeps PE free, exact same zeros as a -inf mask).
  - PV uses P-stationary matmuls: lhsT = P^T block [k=128, q=128],
    rhs = V' [k=128, 65] (64 V cols + ones col). Output lands
    [q-partitions, d | denom] in PSUM, so the softmax denominator is a
    per-partition scalar: reciprocal + tensor_scalar_mul on DVE/Pool,
    no partition broadcasts or DMA hops.
  - O [q,d] bf16 is transposed back to O^T [d,q] pair-tiles with PE
    transposes; head A targets PSUM partitions 0-63, head B 64-127
    (col-tiling), so one DVE copy refills the pair tile.
  - Output projection per 512-token group is emitted right after that
    group's attention, overlapping the next group's work.
"""
import os
import sys

sys.path.insert(0, "/opt/trn_rl_repo")

import numpy as np

import concourse.bacc as bacc
import concourse.mybir as mybir
import concourse.tile as tile
from concourse.bass_utils import run_bass_kernel_spmd

B, T, C = 4, 2048, 1024
H = 16
HD = C // H              # 64
N_CORES = 8
HL = H // 2              # 8 local heads per core
CL = HL * HD             # 512 local channels
F32 = mybir.dt.float32
BF16 = mybir.dt.bfloat16

QG = 512                 # q-group width (4 groups)
NQG = T // QG            # 4
KB = 128                 # k-block
NTCH = T // 128          # 16 t-chunks (V' tiles)
NCCH = C // 128          # 8 contraction chunks

_cache = {}


def _build(dbg=False, reps=1):
    nc = bacc.Bacc("TRN2", target_bir_lowering=False, debug=False,
                   num_devices=N_CORES)

    xT = nc.dram_tensor("xT", [C, T], BF16, kind="ExternalInput")
    wqk = nc.dram_tensor("wqk", [C, 2 * CL], BF16, kind="ExternalInput")
    wv = nc.dram_tensor("wv", [C, CL], BF16, kind="ExternalInput")
    wout = nc.dram_tensor("wout", [CL, C], BF16, kind="ExternalInput")
    tmask = nc.dram_tensor("tmask", [KB, KB], BF16, kind="ExternalInput")
    ident = nc.dram_tensor("ident", [KB, KB], BF16, kind="ExternalInput")
    y = nc.dram_tensor("y", [T, C], F32, kind="ExternalOutput")

    with tile.TileContext(nc) as tc:
      for _rep in range(reps):
        with tc.tile_pool(name="persist", bufs=1) as pp:
            # ---- persistent SBUF tiles ----
            qk = [pp.tile([128, T], BF16, tag=f"qk{j}", name=f"qk{j}")
                  for j in range(8)]
            #   qk[0..3] = Q^T pairs (head 2j at parts 0-63, 2j+1 at 64-127)
            #   qk[4..7] = K^T pairs
            vws = [pp.tile([128, HL * (HD + 1)], BF16, tag=f"vw{m}",
                           name=f"vw{m}") for m in range(NTCH)]
            ot = [pp.tile([128, T], BF16, tag=f"ot{j}", name=f"ot{j}")
                  for j in range(4)]
            wo = [pp.tile([128, C], BF16, tag=f"wo{j}", name=f"wo{j}")
                  for j in range(4)]
            tm = pp.tile([KB, KB], BF16, tag="tm", name="tm")
            idn = pp.tile([KB, KB], BF16, tag="idn", name="idn")
            # phase-1 input tiles (single big DMAs, chunk-sliced in SBUF)
            xt = pp.tile([128, NCCH * T], BF16, tag="xt", name="xt")
            wq = pp.tile([128, NCCH * 2 * CL], BF16, tag="wq", name="wq")
            wvt = pp.tile([128, NCCH * CL], BF16, tag="wvt", name="wvt")

            nc.sync.dma_start(tm[:], tmask[:])
            nc.sync.dma_start(idn[:], ident[:])
            nc.sync.dma_start(
                xt[:].rearrange("p (i t) -> p i t", i=NCCH),
                xT[:].rearrange("(i p) t -> p i t", i=NCCH))
            nc.sync.dma_start(
                wq[:].rearrange("p (i n) -> p i n", i=NCCH),
                wqk[:].rearrange("(i p) n -> p i n", i=NCCH))
            nc.sync.dma_start(
                wvt[:].rearrange("p (i n) -> p i n", i=NCCH),
                wv[:].rearrange("(i p) n -> p i n", i=NCCH))
            nc.sync.dma_start(
                wo[0][:], wout[0 * 128:1 * 128, :])
            nc.sync.dma_start(
                wo[1][:], wout[1 * 128:2 * 128, :])
            nc.sync.dma_start(
                wo[2][:], wout[2 * 128:3 * 128, :])
            nc.sync.dma_start(
                wo[3][:], wout[3 * 128:4 * 128, :])

            def xt_s(i, t0, t1):
                return xt[:, i * T + t0: i * T + t1]

            def wq_s(i, c0, c1):
                return wq[:, i * 2 * CL + c0: i * 2 * CL + c1]

            def wv_s(i):
                return wvt[:, i * CL:(i + 1) * CL]

            with (
                tc.tile_pool(name="ps_mm", bufs=1, space="PSUM") as ps_mm,
                tc.tile_pool(name="ps_s", bufs=2, space="PSUM") as ps_s,
                tc.tile_pool(name="ps_o", bufs=4, space="PSUM") as ps_o,
                tc.tile_pool(name="ps_t", bufs=1, space="PSUM") as ps_t,
                tc.tile_pool(name="pb", bufs=34) as pb,
                tc.tile_pool(name="pn", bufs=4) as pn,
                tc.tile_pool(name="py", bufs=2) as py,
            ):
                cp_eng = [nc.vector, nc.gpsimd]

                def p1_qk(tq, j):
                    # Q^T/K^T columns j*128..+128 for tokens tq*512..+512
                    t0 = tq * QG
                    ps = ps_mm.tile([128, QG], F32, tag="mm", name="mm")
                    for i in range(NCCH):
                        nc.tensor.matmul(
                            ps[:], wq_s(i, j * 128, (j + 1) * 128),
                            xt_s(i, t0, t0 + QG),
                            start=(i == 0), stop=(i == NCCH - 1))
                    nc.vector.tensor_copy(qk[j][:, t0:t0 + QG], ps[:])

                def p1_v(tq, m):
                    # V' tile for t-chunk m (128 tokens) of quarter tq
                    mm = tq * 4 + m
                    t0 = mm * 128
                    ps = ps_mm.tile([128, CL], F32, tag="mm", name="mmv")
                    for i in range(NCCH):
                        nc.tensor.matmul(
                            ps[:], xt_s(i, t0, t0 + 128), wv_s(i),
                            start=(i == 0), stop=(i == NCCH - 1))
                    vt = vws[mm][:].rearrange("p (h x) -> p h x", x=HD + 1)
                    nc.vector.tensor_copy(
                        vt[:, :, 0:HD],
                        ps[:].rearrange("p (h d) -> p h d", d=HD))
                    nc.vector.memset(vt[:, :, HD:HD + 1], 1.0)

                def p1_quarter(tq):
                    for j in range(8):
                        p1_qk(tq, j)
                    for m in range(4):
                        p1_v(tq, m)

                def p1_slice(tq, pair):
                    p1_qk(tq, 2 * pair)
                    p1_qk(tq, 2 * pair + 1)
                    p1_v(tq, pair)

                def attn(g, pair):
                    qlo = g * QG
                    nkb = 4 * g + 4
                    # ---- S + exp for every k-block (P tiles kept live) ----
                    p_all = [[], []]
                    for kb in range(nkb):
                        r0 = max(0, kb * KB - qlo)
                        for x in range(2):   # head A (parts 0-63) / B
                            pb0 = x * 64
                            s_ps = ps_s.tile([128, QG], F32, tag="sps",
                                             name=f"sps{x}")
                            nc.tensor.matmul(
                                s_ps[:, r0:QG],
                                qk[4 + pair][pb0:pb0 + 64,
                                             kb * KB:(kb + 1) * KB],
                                qk[pair][pb0:pb0 + 64,
                                         qlo + r0:qlo + QG],
                                start=True, stop=True)
                            p_x = pb.tile([128, QG], BF16, tag="p",
                                          name=f"p{x}k{kb}")
                            nc.scalar.activation(
                                p_x[:, r0:QG], s_ps[:, r0:QG],
                                mybir.ActivationFunctionType.Exp,
                                scale=0.125)
                            if kb >= 4 * g:   # diagonal block: mask
                                cp_eng[(kb + x) % 2].tensor_mul(
                                    p_x[:, r0:r0 + KB],
                                    p_x[:, r0:r0 + KB], tm[:])
                            p_all[x].append(p_x)
                    # ---- PV' per q-block: one accumulation group at a
                    # time per PSUM bank (sequential starts), normalize +
                    # transpose between groups ----
                    tp = ps_t.tile([128, QG], BF16, tag="tp", name="tp")
                    rcp = [pn.tile([128, 4], F32, tag="rcp", name=f"rcp{x}")
                           for x in range(2)]
                    ob = [pn.tile([128, 4 * HD], BF16, tag="ob",
                                  name=f"ob{x}") for x in range(2)]
                    for qb in range(4):
                        o_t = [ps_o.tile([128, HD + 1], F32, tag="ops",
                                         name=f"ops{x}q{qb}")
                               for x in range(2)]
                        for x in range(2):
                            h = 2 * pair + x
                            for kb in range(4 * g + qb + 1):
                                vsl = vws[kb][:, h * (HD + 1):
                                              (h + 1) * (HD + 1)]
                                nc.tensor.matmul(
                                    o_t[x][:],
                                    p_all[x][kb][:, qb * KB:(qb + 1) * KB],
                                    vsl,
                                    start=(kb == 0),
                                    stop=(kb == 4 * g + qb))
                        for x in range(2):
                            nc.vector.reciprocal(
                                rcp[x][:, qb:qb + 1],
                                o_t[x][:, HD:HD + 1])
                            nc.vector.tensor_scalar_mul(
                                ob[x][:, qb * HD:(qb + 1) * HD],
                                o_t[x][:, 0:HD],
                                rcp[x][:, qb:qb + 1])
                            nc.tensor.transpose(
                                tp[x * 64:x * 64 + 64,
                                   qb * KB:(qb + 1) * KB],
                                ob[x][:, qb * HD:(qb + 1) * HD],
                                idn[:])
                    nc.vector.tensor_copy(ot[pair][:, qlo:qlo + QG], tp[:])

                def proj(g):
                    qlo = g * QG
                    ys = py.tile([128, 4 * C], F32, tag="ys", name="ys")
                    for tc_ in range(4):
                        t0 = qlo + tc_ * 128
                        for n in range(2):
                            ps = ps_mm.tile([128, 512], F32, tag="mm",
                                            name="mmp")
                            for j in range(4):
                                nc.tensor.matmul(
                                    ps[:],
                                    ot[j][:, t0:t0 + 128],
                                    wo[j][:, n * 512:(n + 1) * 512],
                                    start=(j == 0), stop=(j == 3))
                            nc.vector.tensor_copy(
                                ys[:, tc_ * C + n * 512:
                                   tc_ * C + (n + 1) * 512], ps[:])
                    nc.sync.dma_start(
                        y[qlo:qlo + QG, :].rearrange(
                            "(tc p) c -> p tc c", tc=4),
                        ys[:].rearrange("p (tc c) -> p tc c", tc=4))

                # ---------------- schedule ----------------
                p1_quarter(0)
                for g in range(NQG):
                    for pair in range(4):
                        attn(g, pair)
                        if g < NQG - 1:
                            p1_slice(g + 1, pair)
                    proj(g)

    nc.compile()
    return nc


def make_in_maps(x, W_qkv, W_out):
    import ml_dtypes
    bf16 = ml_dtypes.bfloat16
    x = np.asarray(x, dtype=np.float32)
    W_qkv = np.asarray(W_qkv, dtype=np.float32)
    W_out = np.asarray(W_out, dtype=np.float32)

    # lower-triangle keep-mask for the diagonal S^T blocks (rows=k, cols=q:
    # keep k<=q) and the identity for PE transposes
    tmask = (np.arange(KB)[:, None] <= np.arange(KB)[None, :]).astype(bf16)
    ident = np.eye(KB).astype(bf16)

    in_maps = []
    for c in range(N_CORES):
        b, h2 = c // 2, c % 2
        cols = slice(h2 * CL, (h2 + 1) * CL)
        in_maps.append({
            "xT": np.ascontiguousarray(x[b].T).astype(bf16),
            "wqk": np.ascontiguousarray(
                np.concatenate([W_qkv[:, cols],
                                W_qkv[:, C:][:, cols]], axis=1)).astype(bf16),
            "wv": np.ascontiguousarray(W_qkv[:, 2 * C:][:, cols]).astype(bf16),
            "wout": np.ascontiguousarray(W_out[cols, :]).astype(bf16),
            "tmask": tmask,
            "ident": ident,
        })
    return in_maps


def kernel(x, W_qkv, b_qkv, W_out, b_out, _trace=False):
    b_qkv = np.asarray(b_qkv, dtype=np.float32)
    b_out = np.asarray(b_out, dtype=np.float32)
    W_out_f = np.asarray(W_out, dtype=np.float32)

    # q/k biases would need device-side adds; this problem pins them to 0.
    assert not b_qkv[:2 * C].any(), "nonzero q/k bias unsupported"

    if "nc" not in _cache:
        _cache["nc"] = _build()
    nc = _cache["nc"]

    in_maps = make_in_maps(x, W_qkv, W_out)

    kwargs = {}
    if _trace:
        kwargs = {"trace": True, "trace_cores": [0]}
    res = run_bass_kernel_spmd(nc, in_maps, core_ids=list(range(N_CORES)),
                               **kwargs)

    out = np.empty((B, T, C), dtype=np.float32)
    # v-bias passes through softmax as +b_v, so it folds into the output
    # projection; b_out likewise. Both are host-side adds on the partials.
    bias = b_qkv[2 * C:] @ W_out_f + b_out
    for b in range(B):
        out[b] = res.results[2 * b]["y"] + res.results[2 * b + 1]["y"] + bias
    if _trace:
        kernel.last_exec_ns = res.exec_time_ns
        kernel.last_trace = (res.instructions_and_trace or (None, None))[1]
    return out
